# revision 1
# baseline (speedup 1.0000x reference)
"""DirGCNConv on 8 Trainium2 NeuronCores (Bass/Tile).

out = 0.5*(A_norm @ x) @ W_sd.T + 0.5*(A_norm.T @ x) @ W_ds.T + 0.5*(b_sd+b_ds)
with A_norm[r,c] = out_deg(r)^-1/2 * in_deg(c)^-1/2 for each edge (r,c).

Strategy (1D node partition, dest-sharded):
- nodes split into 8 shards of 6250 dests; core p computes out rows of shard p
- x is replicated in each core's HBM as two fp16 tables (rows 0..24999 /
  25000..49999, because dma_gather indices are int16)
- per core, per direction, edges incident to its dests are sorted by local
  dest, grouped into dest blocks of 128, split by source half, padded to
  128-edge tiles; pad slots gather row 0 with weight 0
- gathers are spread round-robin over 4 SWDGE queues so all four Q7
  descriptor-generation cpu pairs run in parallel (a single queue leaves
  3/4 of the GpSimd cluster idle and serializes on its DGE ring)
- per 128-edge tile: S[e, d] = w_e * (iota_d == doff_e) built on DVE
  (tensor_scalar is_eq+mult) or the Activation engine (Abs then
  Relu(w - w*t)) so neither engine is the serial bottleneck;
  PE accumulates matmul(psum, lhsT=M, rhs=S) -> psum[f, d] per dest block.
  aggT layout [f, 6250] feeds the final linear directly.
- final: per 128-dest chunk, psum[d, fo] = aggT_sd[:,chunk].T @ (0.5 W_sd.T)
  + aggT_ds[:,chunk].T @ (0.5 W_ds.T); add bias; DMA to out.

The program is SPMD-uniform: tile counts per (dir, block, half) cell are the
max over cores, padded with (idx=0, w=0) edges.
"""
import os
import sys
import types

sys.path.insert(0, "/opt/trn_rl_repo")
sys.path.insert(0, "/root/.axon_site")

import numpy as np

N = 50000
E = 625000
D = 128
NCORES = 8
SHARD = N // NCORES            # 6250
NBLK = (SHARD + 127) // 128    # 49
HALF = 25000
ALPHA = 0.5

GT = os.environ.get("KERNEL_GT", "float16")   # gather-table / matmul dtype
G_BLOCKS = int(os.environ.get("KERNEL_GBLK", "2"))  # dest blocks per group
GMAX_TILES = int(os.environ.get("KERNEL_GMAX", "15"))  # tiles per gather call
NQUEUES = int(os.environ.get("KERNEL_NQ", "4"))      # SWDGE queues (1..4)
ACT_EVERY = int(os.environ.get("KERNEL_ACT", "4"))   # every Nth S-build on Act

LAST_EXEC_NS = None


def _np_gt():
    return {"float32": np.float32, "float16": np.float16}[GT]


def _install_ntff_hook():
    try:
        import trn_agent_boot.trn_boot as tb
        mod = types.ModuleType("antenv.axon_hooks")
        _hook = [tb._ntff_profile_via_ctypes('/opt/axon/libaxon_pjrt.so')]
        mod.set_axon_ntff_profile_hook = lambda h: _hook.__setitem__(0, h)
        mod.get_axon_ntff_profile_hook = lambda: _hook[0]
        sys.modules["antenv.axon_hooks"] = mod
        return True
    except Exception:
        return False


def _split_excess_waits(nc, mybir, keep=1):
    """Move excess sync waits onto preceding same-engine NoOps (walrus only
    accepts a limited number of sync-wait commands per instruction)."""
    import bass_rust
    k = 0
    for fn in nc.m.functions:
        for bb in fn.blocks:
            out = []
            changed = False
            for inst in bb.instructions:
                si = inst.sync_info
                waits = list(si.on_wait) if si is not None else []
                if len(waits) > keep:
                    changed = True
                    excess, last = waits[:-keep], waits[-keep:]
                    for w in excess:
                        nop = mybir.InstNoOp(
                            name=f"waitnop-{k}", ins=[], outs=[], engine=inst.engine
                        )
                        k += 1
                        nop.sync_info = bass_rust.SyncInfo(on_wait=[w], on_update=[])
                        nc.register_instruction(nop, overwrite=True)
                        out.append(nop)
                    inst.sync_info = bass_rust.SyncInfo(
                        on_wait=last, on_update=list(si.on_update)
                    )
                out.append(inst)
            if changed:
                bb.instructions = out
    return k


def _plan_and_pack(edge_index, w):
    """Host-side edge partition. Returns the (core-uniform) plan plus per-core
    packed idx/doff/wgt arrays.

    Canonical tile order: dir -> block-group -> half -> block -> tile.
    Gather groups are chunks of <=GMAX_TILES within a (dir, bg, half) run.
    """
    row, col = edge_index[0].astype(np.int64), edge_index[1].astype(np.int64)

    # per (dir, core): local-dest-sorted edge arrays
    per = {}   # (dir, core) -> (d_local, src, wv) sorted by d_local
    for di, (dst, src) in enumerate(((row, col), (col, row))):
        shard_of = dst // SHARD
        order = np.argsort(dst, kind="stable")
        dsts, srcs, ws_, sh = dst[order], src[order], w[order], shard_of[order]
        starts = np.searchsorted(sh, np.arange(NCORES + 1))
        for p in range(NCORES):
            s, e = starts[p], starts[p + 1]
            per[(di, p)] = (dsts[s:e] - p * SHARD, srcs[s:e], ws_[s:e])

    # cell edge lists: cells[(dir, half, blk)][core] = (doff, src_local, w)
    cells = {}
    for (di, p), (dl, sl, wl) in per.items():
        blk = dl // 128
        half = (sl >= HALF).astype(np.int64)
        key = blk * 2 + half
        order = np.argsort(key, kind="stable")
        dl, sl, wl, key = dl[order], sl[order], wl[order], key[order]
        bounds = np.searchsorted(key, np.arange(2 * NBLK + 1))
        for b in range(NBLK):
            for h in (0, 1):
                s, e = bounds[b * 2 + h], bounds[b * 2 + h + 1]
                cells.setdefault((di, h, b), {})[p] = (
                    (dl[s:e] - b * 128).astype(np.float32),
                    (sl[s:e] - h * HALF).astype(np.int64),
                    wl[s:e].astype(np.float32),
                )

    # uniform tile counts; lo half >= 1 tile so every block has >=1 matmul
    T_cell = {}
    for (di, h, b), by_core in cells.items():
        mx = max(len(v[0]) for v in by_core.values())
        t = (mx + 127) // 128
        if h == 0:
            t = max(t, 1)
        T_cell[(di, h, b)] = t

    # block groups of G_BLOCKS consecutive blocks
    bgs = [list(range(i, min(i + G_BLOCKS, NBLK))) for i in range(0, NBLK, G_BLOCKS)]

    # canonical tile enumeration + gather groups (chunks of <=GMAX_TILES
    # within each (dir, bg, half) run; one source table per group)
    groups = []   # dicts(dir, bg_index, half, t0, ntiles)
    tile_of = {}  # (di, h, b, k) -> global tile idx
    t = 0
    for di in (0, 1):
        for gi, bg in enumerate(bgs):
            for h in (0, 1):
                run0 = t
                for b in bg:
                    for k in range(T_cell[(di, h, b)]):
                        tile_of[(di, h, b, k)] = t
                        t += 1
                nrun = t - run0
                o = 0
                while o < nrun:
                    take = min(GMAX_TILES, nrun - o)
                    groups.append(dict(dir=di, bg=gi, half=h,
                                       t0=run0 + o, ntiles=take))
                    o += take
    T_total = t

    # per-core packed arrays; pads: idx=0, doff=0, w=0 (row 0 gathered,
    # zeroed by S)
    idx_all, doff_all, wgt_all = [], [], []
    for p in range(NCORES):
        idx16 = np.zeros((T_total * 128,), np.int16)
        doff = np.zeros((T_total * 128,), np.float32)
        wgt = np.zeros((T_total * 128,), np.float32)
        for di in (0, 1):
            for h in (0, 1):
                for b in range(NBLK):
                    dl, sl, wl = cells[(di, h, b)][p]
                    n = len(dl)
                    o = tile_of[(di, h, b, 0)] * 128
                    idx16[o:o + n] = sl.astype(np.int16)
                    doff[o:o + n] = dl
                    wgt[o:o + n] = wl
        # pack: idx [128, T_total*8] (16-wrap, 8x replicated);
        # doff/wgt [128, T_total]
        idx_p = np.tile(idx16.reshape(-1, 16).T, (8, 1)).copy()
        doff_p = doff.reshape(-1, 128).T.copy()
        wgt_p = wgt.reshape(-1, 128).T.copy()
        idx_all.append(idx_p)
        doff_all.append(doff_p)
        wgt_all.append(wgt_p)

    plan = dict(T_cell=T_cell, bgs=bgs, groups=groups, T_total=T_total,
                tile_of=tile_of)
    return plan, idx_all, doff_all, wgt_all


def _build_program(plan):
    from concourse import bacc, tile, mybir

    dt_gt = {"float32": mybir.dt.float32, "float16": mybir.dt.float16}[GT]
    T_cell, bgs, groups, T_total, tile_of = (
        plan["T_cell"], plan["bgs"], plan["groups"], plan["T_total"],
        plan["tile_of"],
    )

    nc = bacc.Bacc(None, target_bir_lowering=False, debug=False,
                   num_swdge_queues=NQUEUES)

    t_xlo = nc.declare_dram_parameter("xlo", [HALF, D], dt_gt, isOutput=False)
    t_xhi = nc.declare_dram_parameter("xhi", [HALF, D], dt_gt, isOutput=False)
    t_idx = nc.declare_dram_parameter("idx", [128, T_total * 8], mybir.dt.int16,
                                      isOutput=False)
    # GT consts: iota (for the Act path)
    t_cgt = nc.declare_dram_parameter("cgt", [128, 128], dt_gt, isOutput=False)
    # f32 consts: Wsd_rhs | Wds_rhs | bias_bcast | doff | wgt | negdoff |
    # negwgt | iota32 | ones (fp32 iota keeps the DVE tensor_scalar on its
    # fast path; the fp16-in variant hits a ~2us/op slow path)
    CF_W = 3 * D + 4 * T_total + 256
    t_cf = nc.declare_dram_parameter("cf32", [128, CF_W], mybir.dt.float32,
                                     isOutput=False)
    t_out = nc.declare_dram_parameter("out", [SHARD, D], mybir.dt.float32,
                                      isOutput=True)

    with tile.TileContext(nc) as tc:
        with (
            tc.tile_pool(name="const", bufs=1) as constp,
            tc.tile_pool(name="agg", bufs=1) as aggp,
            tc.tile_pool(name="gat", bufs=12) as gatp,
            tc.tile_pool(name="s", bufs=32) as sp,
            tc.tile_pool(name="acttmp", bufs=8) as actp,
            tc.tile_pool(name="outp", bufs=8) as outp,
            tc.tile_pool(name="psum", bufs=4, space="PSUM") as psump,
            tc.tile_pool(name="psumo", bufs=2, space="PSUM") as psumop,
            tc.tile_pool(name="psumj", bufs=1, space="PSUM") as psumjp,
        ):
            idx_sb = constp.tile([128, T_total * 8], mybir.dt.int16, tag="idx")
            cgt_sb = constp.tile([128, 128], dt_gt, tag="cgt")
            cf_sb = constp.tile([128, CF_W], mybir.dt.float32, tag="cf")
            nc.sync.dma_start(out=idx_sb[:, 0:128], in_=t_idx[:, 0:128])
            nc.sync.dma_start(out=idx_sb[:, 128:], in_=t_idx[:, 128:])
            nc.sync.dma_start(out=cgt_sb[:], in_=t_cgt[:])
            nc.sync.dma_start(out=cf_sb[:], in_=t_cf[:])
            iota_sb = cgt_sb
            w1_sb = cf_sb[:, 0:D]
            w2_sb = cf_sb[:, D:2 * D]
            bias_sb = cf_sb[:, 2 * D:3 * D]
            o3 = 3 * D
            doff_sb = cf_sb[:, o3:o3 + T_total]
            wgt_sb = cf_sb[:, o3 + T_total:o3 + 2 * T_total]
            ndoff_sb = cf_sb[:, o3 + 2 * T_total:o3 + 3 * T_total]
            nwgt_sb = cf_sb[:, o3 + 3 * T_total:o3 + 4 * T_total]
            iota32_sb = cf_sb[:, o3 + 4 * T_total:o3 + 4 * T_total + 128]
            ones_sb = cf_sb[:, o3 + 4 * T_total + 128:o3 + 4 * T_total + 256]

            psum_junk = psumjp.tile([1, 2], mybir.dt.float32, tag="pj")
            # PE observes the const DMA lanes
            nc.tensor.matmul(psum_junk[:1, 0:1], cf_sb[:, 0:1], cf_sb[:, 0:1])
            nc.tensor.matmul(psum_junk[:1, 1:2], cgt_sb[:, 0:1], cgt_sb[:, 0:1])

            aggT_sd = aggp.tile([128, NBLK * 128], mybir.dt.float32, tag="aggT0")
            aggT_ds = aggp.tile([128, NBLK * 128], mybir.dt.float32, tag="aggT1")
            aggT = [aggT_sd, aggT_ds]

            # index gather groups by (dir, bg)
            by_key = {}
            for g in groups:
                by_key.setdefault((g["dir"], g["bg"]), []).append(g)

            n_gather = 0
            n_s = 0
            for gi, bg in enumerate(bgs):
                for di in (0, 1):
                    gtiles = []  # (tile handle, t0, ntiles, observed?)
                    for g in by_key[(di, gi)]:
                        t0, nt = g["t0"], g["ntiles"]
                        gt_t = gatp.tile([128, nt, D], dt_gt, tag="g")
                        src = t_xlo if g["half"] == 0 else t_xhi
                        n = nt * 128
                        nc.gpsimd.dma_gather(
                            gt_t[:], src[:],
                            idx_sb[:, t0 * 8:(t0 + nt) * 8],
                            n, n, D, single_packet=False,
                            queue_num=n_gather % NQUEUES,
                        )
                        n_gather += 1
                        gtiles.append([gt_t, t0, nt, False])

                    # ---- S-builds for the whole (dir, bg) unit up front so
                    # the PE accumulation chain never waits on DVE/Act ----
                    s_of = {}
                    for b in bg:
                        for h in (0, 1):
                            for k in range(T_cell[(di, h, b)]):
                                tg = tile_of[(di, h, b, k)]
                                s_t = sp.tile([128, 128], dt_gt, tag="s")
                                if ACT_EVERY and (n_s % ACT_EVERY
                                                  == ACT_EVERY - 1):
                                    tmp = actp.tile([128, 128], dt_gt, tag="t")
                                    nc.scalar.activation(
                                        tmp[:], iota_sb[:],
                                        mybir.ActivationFunctionType.Abs,
                                        bias=ndoff_sb[:, tg:tg + 1], scale=1.0,
                                    )
                                    nc.scalar.activation(
                                        s_t[:], tmp[:],
                                        mybir.ActivationFunctionType.Relu,
                                        bias=wgt_sb[:, tg:tg + 1],
                                        scale=nwgt_sb[:, tg:tg + 1],
                                    )
                                else:
                                    nc.vector.tensor_scalar(
                                        s_t[:], iota32_sb[:],
                                        doff_sb[:, tg:tg + 1],
                                        wgt_sb[:, tg:tg + 1],
                                        mybir.AluOpType.is_equal,
                                        mybir.AluOpType.mult,
                                    )
                                n_s += 1
                                s_of[tg] = s_t

                    # ---- matmul accumulation chains ----
                    for b in bg:
                        n_mm = T_cell[(di, 0, b)] + T_cell[(di, 1, b)]
                        psum = psump.tile([128, 128], mybir.dt.float32, tag="ps")
                        mm = 0
                        for h in (0, 1):
                            for k in range(T_cell[(di, h, b)]):
                                tg = tile_of[(di, h, b, k)]
                                for ge in gtiles:
                                    if ge[1] <= tg < ge[1] + ge[2]:
                                        # rely on Tile's dep tracking for the
                                        # gather->matmul wait (excess sem waits
                                        # are split onto NoOps after compile)
                                        gt_t, loc = ge[0], tg - ge[1]
                                        break
                                else:
                                    raise AssertionError("tile not found")
                                nc.tensor.matmul(
                                    psum[:], gt_t[:, loc, :], s_of[tg][:],
                                    start=(mm == 0), stop=(mm == n_mm - 1),
                                )
                                mm += 1
                        wc = min(128, SHARD - b * 128)
                        nc.vector.tensor_copy(
                            aggT[di][:, b * 128:b * 128 + wc], psum[:, :wc]
                        )

                # ---- final linear for this block group (both dirs done);
                # bias added via a rank-1 matmul (ones_row x bias_row) ----
                for b in bg:
                    c0 = b * 128
                    wc = min(128, SHARD - c0)
                    pso = psumop.tile([128, D], mybir.dt.float32, tag="po")
                    nc.tensor.matmul(pso[:wc, :], ones_sb[0:1, :wc],
                                     bias_sb[0:1, :], start=True, stop=False)
                    nc.tensor.matmul(pso[:wc, :], aggT[0][:, c0:c0 + wc], w1_sb[:],
                                     start=False, stop=False)
                    nc.tensor.matmul(pso[:wc, :], aggT[1][:, c0:c0 + wc], w2_sb[:],
                                     start=False, stop=True)
                    o_t = outp.tile([128, D], mybir.dt.float32, tag="o")
                    nc.vector.tensor_copy(o_t[:wc, :], pso[:wc, :])
                    nc.sync.dma_start(out=t_out[c0:c0 + wc, :], in_=o_t[:wc, :])

    nc.compile()
    nsplit = _split_excess_waits(nc, __import__("concourse.mybir", fromlist=["x"]))
    if os.environ.get("KERNEL_VERBOSE"):
        print(f"[kernel] split {nsplit} excess waits; T_total={T_total}, "
              f"groups={len(groups)}")
    return nc


def _prepare(x, edge_index, W_sd, b_sd, W_ds, b_ds):
    """Host preprocessing + program build. Returns (nc, in_maps)."""
    x = np.asarray(x, np.float32)
    edge_index = np.asarray(edge_index, np.int32)
    W_sd = np.asarray(W_sd, np.float32)
    b_sd = np.asarray(b_sd, np.float32)
    W_ds = np.asarray(W_ds, np.float32)
    b_ds = np.asarray(b_ds, np.float32)

    # ---- degrees / edge weights (host) ----
    row, col = edge_index[0].astype(np.int64), edge_index[1].astype(np.int64)
    out_deg = np.bincount(row, minlength=N).astype(np.float32)
    in_deg = np.bincount(col, minlength=N).astype(np.float32)
    out_inv = np.where(out_deg > 0, 1.0 / np.sqrt(np.maximum(out_deg, 1)), 0.0)
    in_inv = np.where(in_deg > 0, 1.0 / np.sqrt(np.maximum(in_deg, 1)), 0.0)
    w = (out_inv[row] * in_inv[col]).astype(np.float32)

    plan, idx_all, doff_all, wgt_all = _plan_and_pack(edge_index, w)

    npgt = _np_gt()
    xlo = np.ascontiguousarray(x[:HALF]).astype(npgt)
    xhi = np.ascontiguousarray(x[HALF:]).astype(npgt)
    iota = np.tile(np.arange(128, dtype=np.float32), (128, 1)).astype(npgt)
    w1 = (ALPHA * W_sd.T).astype(np.float32).copy()
    w2 = ((1.0 - ALPHA) * W_ds.T).astype(np.float32).copy()
    bias = (ALPHA * b_sd + (1.0 - ALPHA) * b_ds).astype(np.float32)
    bias_bc = np.tile(bias, (128, 1)).copy()

    nc = _build_program(plan)

    in_maps = []
    for p in range(NCORES):
        # fp16-round the weights used on the DVE path so both engines' S
        # tiles match the same fp16 values
        wgt16 = wgt_all[p].astype(npgt).astype(np.float32)
        iota32 = np.tile(np.arange(128, dtype=np.float32), (128, 1))
        ones128 = np.ones((128, 128), dtype=np.float32)
        cf32 = np.concatenate(
            [w1, w2, bias_bc, doff_all[p], wgt16, -doff_all[p], -wgt16,
             iota32, ones128],
            axis=1,
        ).astype(np.float32)
        in_maps.append({
            "xlo": xlo, "xhi": xhi,
            "idx": idx_all[p],
            "cgt": iota,
            "cf32": cf32,
        })
    return nc, in_maps


def kernel(x, edge_index, W_sd, b_sd, W_ds, b_ds):
    global LAST_EXEC_NS
    nc, in_maps = _prepare(x, edge_index, W_sd, b_sd, W_ds, b_ds)

    from concourse.bass_utils import run_bass_kernel_spmd

    want_trace = bool(os.environ.get("KERNEL_TRACE"))
    if want_trace:
        want_trace = _install_ntff_hook()
    core_ids = list(range(NCORES))
    res = run_bass_kernel_spmd(nc, in_maps, core_ids, trace=want_trace)
    LAST_EXEC_NS = res.exec_time_ns

    out = np.concatenate([res.results[p]["out"] for p in range(NCORES)], axis=0)
    return out.astype(np.float32)



# revision 4
# speedup vs baseline: 2.2249x; 2.2249x over previous
"""DirGCNConv on 8 Trainium2 NeuronCores (Bass/Tile).

out = 0.5*(A_norm @ x) @ W_sd.T + 0.5*(A_norm.T @ x) @ W_ds.T + 0.5*(b_sd+b_ds)
with A_norm[r,c] = out_deg(r)^-1/2 * in_deg(c)^-1/2 for each edge (r,c).

Strategy (1D node partition, dest-sharded):
- nodes split into 8 shards of 6250 dests; core p computes out rows of shard p
- x is replicated in each core's HBM as two fp16 tables (rows 0..24999 /
  25000..49999, because dma_gather indices are int16)
- per core, per direction, edges incident to its dests are sorted by local
  dest, grouped into dest blocks of 128, split by source half, padded to
  128-edge tiles; pad slots gather row 0 with weight 0
- gathers are spread round-robin over 4 SWDGE queues
- per 128-edge tile the selection matrix S[e, d] = w_e * (d == doff_e) is
  PRECOMPUTED ON HOST and streamed from HBM (one dma_start per (dir, bg)
  unit), so DVE/Act do no per-tile work and the only per-edge on-chip cost
  is the SWDGE gather itself.
  PE accumulates matmul(psum, lhsT=M, rhs=S) -> psum[f, d] per dest block.
  aggT layout [f, 6250] feeds the final linear directly.
- final: per 128-dest chunk, psum[d, fo] = aggT_sd[:,chunk].T @ (0.5 W_sd.T)
  + aggT_ds[:,chunk].T @ (0.5 W_ds.T); add bias; DMA to out.

The program is SPMD-uniform: tile counts per (dir, block, half) cell are the
max over cores, padded with (idx=0, w=0) edges.
"""
import os
import sys
import types

sys.path.insert(0, "/opt/trn_rl_repo")
sys.path.insert(0, "/root/.axon_site")

import numpy as np

N = 50000
E = 625000
D = 128
NCORES = 8
SHARD = N // NCORES            # 6250
NBLK = (SHARD + 127) // 128    # 49
HALF = 25000
ALPHA = 0.5

GT = os.environ.get("KERNEL_GT", "float16")   # gather-table / matmul dtype
G_BLOCKS = int(os.environ.get("KERNEL_GBLK", "2"))  # dest blocks per group
GMAX_TILES = int(os.environ.get("KERNEL_GMAX", "15"))  # tiles per gather call
NQUEUES = int(os.environ.get("KERNEL_NQ", "4"))      # SWDGE queues (1..4)
SINGLE_PACKET = bool(int(os.environ.get("KERNEL_SP", "0")))
GATBUFS = int(os.environ.get("KERNEL_GATBUFS", "12"))
SBUFS = int(os.environ.get("KERNEL_SBUFS", "4"))

LAST_EXEC_NS = None


def _np_gt():
    return {"float32": np.float32, "float16": np.float16}[GT]


def _install_ntff_hook():
    try:
        import trn_agent_boot.trn_boot as tb
        mod = types.ModuleType("antenv.axon_hooks")
        _hook = [tb._ntff_profile_via_ctypes('/opt/axon/libaxon_pjrt.so')]
        mod.set_axon_ntff_profile_hook = lambda h: _hook.__setitem__(0, h)
        mod.get_axon_ntff_profile_hook = lambda: _hook[0]
        sys.modules["antenv.axon_hooks"] = mod
        return True
    except Exception:
        return False


def _split_excess_waits(nc, mybir, keep=1):
    """Move excess sync waits onto preceding same-engine NoOps (walrus only
    accepts a limited number of sync-wait commands per instruction)."""
    import bass_rust
    k = 0
    for fn in nc.m.functions:
        for bb in fn.blocks:
            out = []
            changed = False
            for inst in bb.instructions:
                si = inst.sync_info
                waits = list(si.on_wait) if si is not None else []
                if len(waits) > keep:
                    changed = True
                    excess, last = waits[:-keep], waits[-keep:]
                    for w in excess:
                        nop = mybir.InstNoOp(
                            name=f"waitnop-{k}", ins=[], outs=[], engine=inst.engine
                        )
                        k += 1
                        nop.sync_info = bass_rust.SyncInfo(on_wait=[w], on_update=[])
                        nc.register_instruction(nop, overwrite=True)
                        out.append(nop)
                    inst.sync_info = bass_rust.SyncInfo(
                        on_wait=last, on_update=list(si.on_update)
                    )
                out.append(inst)
            if changed:
                bb.instructions = out
    return k


def _plan_and_pack(edge_index, w):
    """Host-side edge partition. Returns the (core-uniform) plan plus per-core
    packed idx/doff/wgt arrays.

    Canonical tile order: dir -> block-group -> half -> block -> tile.
    Gather groups are chunks of <=GMAX_TILES within a (dir, bg, half) run.
    """
    row, col = edge_index[0].astype(np.int64), edge_index[1].astype(np.int64)

    # per (dir, core): local-dest-sorted edge arrays
    per = {}   # (dir, core) -> (d_local, src, wv) sorted by d_local
    for di, (dst, src) in enumerate(((row, col), (col, row))):
        shard_of = dst // SHARD
        order = np.argsort(dst, kind="stable")
        dsts, srcs, ws_, sh = dst[order], src[order], w[order], shard_of[order]
        starts = np.searchsorted(sh, np.arange(NCORES + 1))
        for p in range(NCORES):
            s, e = starts[p], starts[p + 1]
            per[(di, p)] = (dsts[s:e] - p * SHARD, srcs[s:e], ws_[s:e])

    # cell edge lists: cells[(dir, half, blk)][core] = (doff, src_local, w)
    cells = {}
    for (di, p), (dl, sl, wl) in per.items():
        blk = dl // 128
        half = (sl >= HALF).astype(np.int64)
        key = blk * 2 + half
        order = np.argsort(key, kind="stable")
        dl, sl, wl, key = dl[order], sl[order], wl[order], key[order]
        bounds = np.searchsorted(key, np.arange(2 * NBLK + 1))
        for b in range(NBLK):
            for h in (0, 1):
                s, e = bounds[b * 2 + h], bounds[b * 2 + h + 1]
                cells.setdefault((di, h, b), {})[p] = (
                    (dl[s:e] - b * 128).astype(np.float32),
                    (sl[s:e] - h * HALF).astype(np.int64),
                    wl[s:e].astype(np.float32),
                )

    # uniform tile counts; lo half >= 1 tile so every block has >=1 matmul
    T_cell = {}
    for (di, h, b), by_core in cells.items():
        mx = max(len(v[0]) for v in by_core.values())
        t = (mx + 127) // 128
        if h == 0:
            t = max(t, 1)
        T_cell[(di, h, b)] = t

    # block groups of G_BLOCKS consecutive blocks
    bgs = [list(range(i, min(i + G_BLOCKS, NBLK))) for i in range(0, NBLK, G_BLOCKS)]

    # canonical tile enumeration + gather groups (chunks of <=GMAX_TILES
    # within each (dir, bg, half) run; one source table per group)
    groups = []   # dicts(dir, bg_index, half, t0, ntiles)
    tile_of = {}  # (di, h, b, k) -> global tile idx
    unit_of = {}  # (di, bg_index) -> (t0, ntiles) contiguous tile run
    t = 0
    for di in (0, 1):
        for gi, bg in enumerate(bgs):
            unit0 = t
            for h in (0, 1):
                run0 = t
                for b in bg:
                    for k in range(T_cell[(di, h, b)]):
                        tile_of[(di, h, b, k)] = t
                        t += 1
                nrun = t - run0
                o = 0
                while o < nrun:
                    take = min(GMAX_TILES, nrun - o)
                    groups.append(dict(dir=di, bg=gi, half=h,
                                       t0=run0 + o, ntiles=take))
                    o += take
            unit_of[(di, gi)] = (unit0, t - unit0)
    T_total = t

    # per-core packed arrays; pads: idx=0, doff=0, w=0 (row 0 gathered,
    # zeroed by S)
    idx_all, doff_all, wgt_all = [], [], []
    for p in range(NCORES):
        idx16 = np.zeros((T_total * 128,), np.int16)
        doff = np.zeros((T_total * 128,), np.float32)
        wgt = np.zeros((T_total * 128,), np.float32)
        for di in (0, 1):
            for h in (0, 1):
                for b in range(NBLK):
                    dl, sl, wl = cells[(di, h, b)][p]
                    n = len(dl)
                    o = tile_of[(di, h, b, 0)] * 128
                    idx16[o:o + n] = sl.astype(np.int16)
                    doff[o:o + n] = dl
                    wgt[o:o + n] = wl
        # pack: idx [128, T_total*8] (16-wrap, 8x replicated);
        # doff/wgt [128, T_total]
        idx_p = np.tile(idx16.reshape(-1, 16).T, (8, 1)).copy()
        doff_p = doff.reshape(-1, 128).T.copy()
        wgt_p = wgt.reshape(-1, 128).T.copy()
        idx_all.append(idx_p)
        doff_all.append(doff_p)
        wgt_all.append(wgt_p)

    plan = dict(T_cell=T_cell, bgs=bgs, groups=groups, T_total=T_total,
                tile_of=tile_of, unit_of=unit_of)
    return plan, idx_all, doff_all, wgt_all


def _build_program(plan):
    from concourse import bacc, tile, mybir

    dt_gt = {"float32": mybir.dt.float32, "float16": mybir.dt.float16}[GT]
    T_cell, bgs, groups, T_total, tile_of, unit_of = (
        plan["T_cell"], plan["bgs"], plan["groups"], plan["T_total"],
        plan["tile_of"], plan["unit_of"],
    )

    nc = bacc.Bacc(None, target_bir_lowering=False, debug=False,
                   num_swdge_queues=NQUEUES)

    t_xlo = nc.declare_dram_parameter("xlo", [HALF, D], dt_gt, isOutput=False)
    t_xhi = nc.declare_dram_parameter("xhi", [HALF, D], dt_gt, isOutput=False)
    t_idx = nc.declare_dram_parameter("idx", [128, T_total * 8], mybir.dt.int16,
                                      isOutput=False)
    # host-precomputed S stream: partition e, col t*128+d = w_e*(d==doff_e)
    t_S = nc.declare_dram_parameter("S", [128, T_total * 128], dt_gt,
                                    isOutput=False)
    # f32 consts: Wsd_rhs | Wds_rhs | bias_bcast | ones
    CF_W = 3 * D + 128
    t_cf = nc.declare_dram_parameter("cf32", [128, CF_W], mybir.dt.float32,
                                     isOutput=False)
    t_out = nc.declare_dram_parameter("out", [SHARD, D], mybir.dt.float32,
                                      isOutput=True)

    with tile.TileContext(nc) as tc:
        with (
            tc.tile_pool(name="const", bufs=1) as constp,
            tc.tile_pool(name="agg", bufs=1) as aggp,
            tc.tile_pool(name="gat", bufs=GATBUFS) as gatp,
            tc.tile_pool(name="s", bufs=SBUFS) as sp,
            tc.tile_pool(name="outp", bufs=8) as outp,
            tc.tile_pool(name="psum", bufs=4, space="PSUM") as psump,
            tc.tile_pool(name="psumo", bufs=2, space="PSUM") as psumop,
            tc.tile_pool(name="psumj", bufs=1, space="PSUM") as psumjp,
        ):
            idx_sb = constp.tile([128, T_total * 8], mybir.dt.int16, tag="idx")
            cf_sb = constp.tile([128, CF_W], mybir.dt.float32, tag="cf")
            nc.sync.dma_start(out=idx_sb[:, 0:128], in_=t_idx[:, 0:128])
            nc.sync.dma_start(out=idx_sb[:, 128:], in_=t_idx[:, 128:])
            nc.sync.dma_start(out=cf_sb[:], in_=t_cf[:])
            w1_sb = cf_sb[:, 0:D]
            w2_sb = cf_sb[:, D:2 * D]
            bias_sb = cf_sb[:, 2 * D:3 * D]
            ones_sb = cf_sb[:, 3 * D:3 * D + 128]

            psum_junk = psumjp.tile([1, 2], mybir.dt.float32, tag="pj")
            # PE observes the const DMA lanes
            nc.tensor.matmul(psum_junk[:1, 0:1], cf_sb[:, 0:1], cf_sb[:, 0:1])

            aggT_sd = aggp.tile([128, NBLK * 128], mybir.dt.float32, tag="aggT0")
            aggT_ds = aggp.tile([128, NBLK * 128], mybir.dt.float32, tag="aggT1")
            aggT = [aggT_sd, aggT_ds]

            # index gather groups by (dir, bg)
            by_key = {}
            for g in groups:
                by_key.setdefault((g["dir"], g["bg"]), []).append(g)

            n_gather = 0
            for gi, bg in enumerate(bgs):
                for di in (0, 1):
                    gtiles = []  # (tile handle, t0, ntiles)
                    for g in by_key[(di, gi)]:
                        t0, nt = g["t0"], g["ntiles"]
                        gt_t = gatp.tile([128, nt, D], dt_gt, tag="g")
                        src = t_xlo if g["half"] == 0 else t_xhi
                        n = nt * 128
                        nc.gpsimd.dma_gather(
                            gt_t[:], src[:],
                            idx_sb[:, t0 * 8:(t0 + nt) * 8],
                            n, n, D, single_packet=SINGLE_PACKET,
                            queue_num=n_gather % NQUEUES,
                        )
                        n_gather += 1
                        gtiles.append([gt_t, t0, nt])

                    # ---- S stream for the whole (dir, bg) unit ----
                    ut0, unt = unit_of[(di, gi)]
                    s_t = sp.tile([128, unt, 128], dt_gt, tag="s")
                    nc.sync.dma_start(
                        out=s_t[:], in_=t_S[:, ut0 * 128:(ut0 + unt) * 128]
                    )

                    # ---- matmul accumulation chains ----
                    for b in bg:
                        n_mm = T_cell[(di, 0, b)] + T_cell[(di, 1, b)]
                        psum = psump.tile([128, 128], mybir.dt.float32, tag="ps")
                        mm = 0
                        for h in (0, 1):
                            for k in range(T_cell[(di, h, b)]):
                                tg = tile_of[(di, h, b, k)]
                                for ge in gtiles:
                                    if ge[1] <= tg < ge[1] + ge[2]:
                                        # rely on Tile's dep tracking for the
                                        # gather->matmul wait (excess sem waits
                                        # are split onto NoOps after compile)
                                        gt_t, loc = ge[0], tg - ge[1]
                                        break
                                else:
                                    raise AssertionError("tile not found")
                                nc.tensor.matmul(
                                    psum[:], gt_t[:, loc, :],
                                    s_t[:, tg - ut0, :],
                                    start=(mm == 0), stop=(mm == n_mm - 1),
                                )
                                mm += 1
                        wc = min(128, SHARD - b * 128)
                        nc.vector.tensor_copy(
                            aggT[di][:, b * 128:b * 128 + wc], psum[:, :wc]
                        )

                # ---- final linear for this block group (both dirs done);
                # bias added via a rank-1 matmul (ones_row x bias_row) ----
                for b in bg:
                    c0 = b * 128
                    wc = min(128, SHARD - c0)
                    pso = psumop.tile([128, D], mybir.dt.float32, tag="po")
                    nc.tensor.matmul(pso[:wc, :], ones_sb[0:1, :wc],
                                     bias_sb[0:1, :], start=True, stop=False)
                    nc.tensor.matmul(pso[:wc, :], aggT[0][:, c0:c0 + wc], w1_sb[:],
                                     start=False, stop=False)
                    nc.tensor.matmul(pso[:wc, :], aggT[1][:, c0:c0 + wc], w2_sb[:],
                                     start=False, stop=True)
                    o_t = outp.tile([128, D], mybir.dt.float32, tag="o")
                    nc.vector.tensor_copy(o_t[:wc, :], pso[:wc, :])
                    nc.sync.dma_start(out=t_out[c0:c0 + wc, :], in_=o_t[:wc, :])

    nc.compile()
    nsplit = _split_excess_waits(nc, __import__("concourse.mybir", fromlist=["x"]))
    if os.environ.get("KERNEL_VERBOSE"):
        print(f"[kernel] split {nsplit} excess waits; T_total={T_total}, "
              f"groups={len(groups)}")
    return nc


def _prepare(x, edge_index, W_sd, b_sd, W_ds, b_ds):
    """Host preprocessing + program build. Returns (nc, in_maps)."""
    x = np.asarray(x, np.float32)
    edge_index = np.asarray(edge_index, np.int32)
    W_sd = np.asarray(W_sd, np.float32)
    b_sd = np.asarray(b_sd, np.float32)
    W_ds = np.asarray(W_ds, np.float32)
    b_ds = np.asarray(b_ds, np.float32)

    # ---- degrees / edge weights (host) ----
    row, col = edge_index[0].astype(np.int64), edge_index[1].astype(np.int64)
    out_deg = np.bincount(row, minlength=N).astype(np.float32)
    in_deg = np.bincount(col, minlength=N).astype(np.float32)
    out_inv = np.where(out_deg > 0, 1.0 / np.sqrt(np.maximum(out_deg, 1)), 0.0)
    in_inv = np.where(in_deg > 0, 1.0 / np.sqrt(np.maximum(in_deg, 1)), 0.0)
    w = (out_inv[row] * in_inv[col]).astype(np.float32)

    plan, idx_all, doff_all, wgt_all = _plan_and_pack(edge_index, w)
    T_total = plan["T_total"]

    npgt = _np_gt()
    xlo = np.ascontiguousarray(x[:HALF]).astype(npgt)
    xhi = np.ascontiguousarray(x[HALF:]).astype(npgt)
    w1 = (ALPHA * W_sd.T).astype(np.float32).copy()
    w2 = ((1.0 - ALPHA) * W_ds.T).astype(np.float32).copy()
    bias = (ALPHA * b_sd + (1.0 - ALPHA) * b_ds).astype(np.float32)
    bias_bc = np.tile(bias, (128, 1)).copy()
    ones128 = np.ones((128, 128), dtype=np.float32)
    cf32 = np.concatenate([w1, w2, bias_bc, ones128], axis=1).astype(np.float32)

    nc = _build_program(plan)

    e_rows = np.arange(128)[:, None]                      # [128, 1]
    t_cols = np.arange(T_total)[None, :] * 128            # [1, T]
    in_maps = []
    for p in range(NCORES):
        S = np.zeros((128, T_total * 128), dtype=npgt)
        cols = t_cols + doff_all[p].astype(np.int64)      # [128, T]
        S[e_rows, cols] = wgt_all[p].astype(npgt)
        in_maps.append({
            "xlo": xlo, "xhi": xhi,
            "idx": idx_all[p],
            "S": S,
            "cf32": cf32,
        })
    return nc, in_maps


def kernel(x, edge_index, W_sd, b_sd, W_ds, b_ds):
    global LAST_EXEC_NS
    nc, in_maps = _prepare(x, edge_index, W_sd, b_sd, W_ds, b_ds)

    from concourse.bass_utils import run_bass_kernel_spmd

    want_trace = bool(os.environ.get("KERNEL_TRACE"))
    if want_trace:
        want_trace = _install_ntff_hook()
    core_ids = list(range(NCORES))
    res = run_bass_kernel_spmd(nc, in_maps, core_ids, trace=want_trace)
    LAST_EXEC_NS = res.exec_time_ns

    out = np.concatenate([res.results[p]["out"] for p in range(NCORES)], axis=0)
    return out.astype(np.float32)
